# revision 22
# baseline (speedup 1.0000x reference)
"""Multihead attention (B=2, S=2048, E=1024, H=16) on 8 TRN2 cores.

Sharding: 2 batch-groups x 4-way head split.  Core c handles batch c//4
and heads {4g..4g+3} where g = c%4 (DOUT = 256 = 2 head-pairs of 128
partition dims).  Each core computes its partial out-projection in bf16;
the host sums the 4 partials per batch and adds the output bias.

Layout / schedule (per core):
  x^T [E, S] streams in bf16 and persists in SBUF.  K-proj and Q-proj
  produce K^T/Q^T [128 = 2 heads x 64, S] per head-pair (contract E on
  partitions, bias via DVE tensor_scalar_add on the PSUM->SBUF move).
  V-proj runs in the transposed orientation (lhsT = x^T chunk, rhs =
  Wv^T) producing V[s, d] chunks directly -- no PE transposes -- with the
  bias folded in as a rank-1 ones x bv matmul; a DVE copy drops them
  into the [V | ones] ctx lhsT ([128 kpos, 65] per head, ones column
  gives the softmax denominator as ctx row 64).
  Attention per (q-block, head-pair): scores for both heads of the pair
  land in one 2-bank PSUM tile [128, 1024]; ONE Exp activation (scale =
  1/sqrt(D)) covers both, halving ACT instruction overhead -- ACT is the
  pacing engine.  ctx accumulates over the 16 kpos tiles in PSUM.
  Normalization: reciprocal_approx_fast on the denominator row (DVE,
  ~5x faster than the exact reciprocal), PE-broadcast via ones64 lhsT
  (f32r bitcast), one DVE multiply into ctxT (bf16).
  Out-proj (contract the 256 local dims, 2 accumulating matmuls) and
  next q-block's Q-proj are interleaved into the attention t-loop so the
  PE never sits behind a phase barrier.
"""

import numpy as np
import ml_dtypes

# Problem constants (hardcoded per the task contract).
B, S, E, H = 2, 2048, 1024, 16
D = E // H            # 64
NCORES = 8
BGROUPS = 2           # batch groups
HSPLIT = NCORES // BGROUPS   # 4-way head split
DOUT = E // HSPLIT    # 256 local head dims = 4 heads = 2 head-pairs
NHP = DOUT // 128     # 2 head-pairs
KE = E // 128         # 8 contraction tiles over E
SEQT = 512            # seq tile for q/k projections and attention q-blocks
NST = S // SEQT       # 4
QB = S // SEQT        # 4 q-blocks
KT = S // 128         # 16 kpos tiles
SC = S // 128         # 16 seq chunks for the V projection
ISD = float(D) ** -0.5

_PROGRAM = None


# ---------------------------------------------------------------------------
# Workarounds for this walrus build: at most ONE sync wait per instruction is
# reliably accepted ("Too many sync wait commands").  (1) tile's final drain
# gets one wait per logical proc — split them over single-wait SP NOPs;
# (2) a general post-pass moves any instruction's excess waits onto
# preceding same-engine NOPs (engine program order preserves semantics).
# ---------------------------------------------------------------------------


def _install_tile_drain_patch():
    import concourse.mybir as mybir
    import concourse.tile as tile
    from concourse.tile import ScopedClock

    if getattr(tile.TileContext, "_drain_patch_installed", False):
        return

    def _patched_drain_and_barrier(self, tick_clock, wait_clock):
        nc = self.nc
        carrier = nc.sync.nop(nofuse=True)
        wait_clock.add_sem_waits(
            carrier.ins, ScopedClock({None: tick_clock.global_clock})
        )
        si = carrier.ins.sync_info
        waits = list(si.on_wait) if si and si.on_wait else []
        ups = list(si.on_update) if si and si.on_update else []
        if len(waits) > 1:
            carrier.ins.sync_info = mybir.SyncInfo(on_wait=[waits[0]], on_update=ups)
            for w in waits[1:]:
                n2 = nc.sync.nop(nofuse=True)
                n2.ins.sync_info = mybir.SyncInfo(on_wait=[w], on_update=[])
        nc.sync.drain()
        nc.all_engine_barrier()
        popped = nc._tile_sem_poison_stack.pop()
        assert popped is self._sem_poison
        nc.clear_and_free_semaphores(list(self.sems.allocated().values()))
        nc.all_engine_barrier()

    tile.TileContext._drain_and_barrier = _patched_drain_and_barrier
    tile.TileContext._drain_patch_installed = True


MAX_WAITS = 1


def _split_excess_waits(nc):
    import concourse.mybir as mybir

    for bb in nc.main_func.blocks:
        il = list(bb.instructions)
        out = []
        changed = False
        for ins in il:
            si = ins.sync_info
            waits = list(si.on_wait) if si and si.on_wait else []
            if len(waits) > MAX_WAITS:
                changed = True
                extras = waits[: len(waits) - MAX_WAITS]
                keep = waits[len(extras):]
                for i in range(0, len(extras), MAX_WAITS):
                    chunk = extras[i : i + MAX_WAITS]
                    nop = mybir.InstNoOp(
                        name=nc.get_next_instruction_name(), ins=[], outs=[]
                    )
                    nop.engine = ins.engine
                    nop.sync_info = mybir.SyncInfo(on_wait=chunk, on_update=[])
                    out.append(nop)
                ins.sync_info = mybir.SyncInfo(
                    on_wait=keep, on_update=list(si.on_update) if si.on_update else []
                )
            out.append(ins)
        if changed:
            bb.instructions = out


def _build_program():
    import concourse.bass as bass
    import concourse.mybir as mybir
    import concourse.tile as tile

    _install_tile_drain_patch()

    f32 = mybir.dt.float32
    f32r = mybir.dt.float32r
    bf16 = mybir.dt.bfloat16
    Exp = mybir.ActivationFunctionType.Exp

    nc = bass.Bass("TRN2", target_bir_lowering=False, debug=False)

    # DRAM I/O (per core).
    xq = nc.dram_tensor("xq", [KE, 128, S], bf16, kind="ExternalInput").ap()
    xk = nc.dram_tensor("xk", [KE, 128, S], bf16, kind="ExternalInput").ap()
    xv = nc.dram_tensor("xv", [KE, 128, S], bf16, kind="ExternalInput").ap()
    wq = nc.dram_tensor("wq", [KE, 128, DOUT], bf16, kind="ExternalInput").ap()
    wk = nc.dram_tensor("wk", [KE, 128, DOUT], bf16, kind="ExternalInput").ap()
    wv = nc.dram_tensor("wv", [KE, 128, DOUT], bf16, kind="ExternalInput").ap()
    wo = nc.dram_tensor("wo", [NHP, 128, E], bf16, kind="ExternalInput").ap()
    bq = nc.dram_tensor("bq", [128, NHP], f32, kind="ExternalInput").ap()
    bk = nc.dram_tensor("bk", [128, NHP], f32, kind="ExternalInput").ap()
    bv = nc.dram_tensor("bv", [1, DOUT], bf16, kind="ExternalInput").ap()
    out = nc.dram_tensor("out", [S, E], bf16, kind="ExternalOutput").ap()

    with tile.TileContext(nc) as tc:
        with (
            nc.allow_low_precision(reason="bf16 attention pipeline"),
            tc.tile_pool(name="consts", bufs=1) as consts,
            tc.tile_pool(name="persist", bufs=1) as persist,
            tc.tile_pool(name="ptp", bufs=3) as ptp,
            tc.tile_pool(name="outp", bufs=4) as outp,
            tc.tile_pool(name="small", bufs=4) as small,
            tc.tile_pool(name="pp_ps", bufs=1, space="PSUM") as pp_ps,
            tc.tile_pool(name="sc_ps", bufs=2, space="PSUM") as sc_ps,
            tc.tile_pool(name="cx_ps", bufs=3, space="PSUM") as cx_ps,
        ):
            # ---- constants ----
            onesf = consts.tile([128, 1], f32)
            nc.vector.memset(onesf[:], 1.0)
            # ones rows (at base partitions 0 and 32) for the reciprocal
            # broadcast matmuls; f32r via copy (memset can't write f32r)
            onesr33 = consts.tile([33, 64], f32r)
            nc.vector.tensor_copy(
                onesr33[:], onesf[0:33, 0:1].broadcast_to([33, 64])
            )
            ones1b = consts.tile([1, 128], bf16)
            nc.vector.memset(ones1b[:], 1.0)

            # ---- persistent weights ----
            wq_sb = persist.tile([128, KE, DOUT], bf16, tag="wq")
            wk_sb = persist.tile([128, KE, DOUT], bf16, tag="wk")
            wv_sb = persist.tile([128, KE, DOUT], bf16, tag="wv")
            wo_sb = persist.tile([128, NHP, E], bf16, tag="wo")
            bq_sb = persist.tile([128, NHP], f32, tag="bq")
            bk_sb = persist.tile([128, NHP], f32, tag="bk")
            bv_sb = persist.tile([1, DOUT], bf16, tag="bv")

            # ---- persistent activations ----
            xq_sb = persist.tile([128, KE, S], bf16, tag="xq")
            xk_sb = persist.tile([128, KE, S], bf16, tag="xk")
            xv_sb = persist.tile([128, KE, S], bf16, tag="xv")
            qt_sb = persist.tile([128, NHP, S], bf16, tag="qt")
            kt_sb = persist.tile([128, NHP, S], bf16, tag="kt")
            # [V | ones] per (kpos chunk, local head): ctx lhsT
            v_sb = persist.tile([128, SC, 4, D + 1], bf16, tag="vn")
            ctxT_sb = persist.tile([128, NHP, S], bf16, tag="ctxT")

            nc.vector.tensor_copy(
                v_sb[:, :, :, D], onesf[:, 0:1].broadcast_to([128, SC, 4])
            )

            # ---- input DMAs ----
            # Trigger dispatch costs ~640ns per dma_start on an engine's
            # queue, so the ~125 triggers are spread across three engines
            # (sync / vector / gpsimd) in first-use order; x tensors are
            # chunked [128, 512] so transfers parallelize across the 16 DMA
            # queues and consumers start on partial data.
            for k in range(KE):          # weights: scalar (idle at start)
                nc.scalar.dma_start(wk_sb[:, k, :], wk[k])
            for k in range(KE):
                nc.scalar.dma_start(wq_sb[:, k, :], wq[k])
            sl0 = bass.ts(0, SEQT)
            for k in range(KE):          # sync: K/Q st0, then K st1..3
                nc.sync.dma_start(xk_sb[:, k, sl0], xk[k, :, sl0])
            for k in range(KE):
                nc.sync.dma_start(xq_sb[:, k, sl0], xq[k, :, sl0])
            nc.sync.dma_start(bk_sb[:], bk[:])
            nc.sync.dma_start(bq_sb[:], bq[:])
            for st in range(1, NST):
                sl = bass.ts(st, SEQT)
                for k in range(KE):
                    nc.sync.dma_start(xk_sb[:, k, sl], xk[k, :, sl])
            # gpsimd: V path, then late Q tiles and the out-proj weights
            for k in range(KE):
                nc.gpsimd.dma_start(wv_sb[:, k, :], wv[k])
            nc.gpsimd.dma_start(bv_sb[:], bv[:])
            for scg in range(2):
                sl = bass.ts(scg, SEQT)
                for k in range(KE):
                    nc.gpsimd.dma_start(xv_sb[:, k, sl], xv[k, :, sl])
            for k in range(KE):
                sl = bass.ts(1, SEQT)
                nc.gpsimd.dma_start(xq_sb[:, k, sl], xq[k, :, sl])
            for scg in range(2, NST):
                sl = bass.ts(scg, SEQT)
                for k in range(KE):
                    nc.gpsimd.dma_start(xv_sb[:, k, sl], xv[k, :, sl])
            for st in range(2, NST):
                sl = bass.ts(st, SEQT)
                for k in range(KE):
                    nc.gpsimd.dma_start(xq_sb[:, k, sl], xq[k, :, sl])
            for hp in range(NHP):
                nc.gpsimd.dma_start(wo_sb[:, hp, :], wo[hp])

            def kq_proj_hp(name, w_sb, x_sb, b_sb, dst, st, hp):
                """One [128, 512] tile of the K or Q projection (transposed
                output layout [d, s]) for one head-pair + bias add."""
                sl = bass.ts(st, SEQT)
                ps = pp_ps.tile([128, SEQT], f32, tag="pp", name=f"{name}{st}{hp}")
                for k in range(KE):
                    nc.tensor.matmul(
                        ps[:],
                        lhsT=w_sb[:, k, bass.ts(hp, 128)],
                        rhs=x_sb[:, k, sl],
                        start=(k == 0),
                        stop=(k == KE - 1),
                    )
                nc.vector.tensor_scalar_add(
                    dst[:, hp, sl], ps[:], b_sb[:, hp : hp + 1]
                )

            def kq_proj(name, w_sb, x_sb, b_sb, dst, st):
                for hp in range(NHP):
                    kq_proj_hp(name, w_sb, x_sb, b_sb, dst, st, hp)

            def v_proj(sc):
                """V rows [128 seq, 256 d] directly via lhsT = x^T chunk;
                bias folded in as a rank-1 (ones x bv) accumulate."""
                ssl = bass.ts(sc, 128)
                ps = pp_ps.tile([128, SEQT], f32, tag="pp", name=f"vp{sc}")
                vp = ps[:, 0:DOUT]
                for k in range(KE):
                    nc.tensor.matmul(
                        vp,
                        lhsT=xv_sb[:, k, ssl],
                        rhs=wv_sb[:, k, :],
                        start=(k == 0),
                        stop=False,
                    )
                nc.tensor.matmul(
                    vp, lhsT=ones1b[:], rhs=bv_sb[:], start=False, stop=True
                )
                for h in range(4):
                    nc.vector.tensor_copy(
                        v_sb[:, sc, h, 0:D], ps[:, bass.ts(h, D)]
                    )

            def outproj_m(qb, m):
                """One output row-tile [128 seq, E]: contract the 256 local
                dims (2 accumulating matmuls per 512-wide chunk), copy to
                SBUF bf16, DMA out."""
                msl = bass.ts(4 * qb + m, 128)
                for n in range(E // SEQT):
                    ps = pp_ps.tile([128, SEQT], f32, tag="pp", name=f"ob{qb}{m}{n}")
                    for hp in range(NHP):
                        nc.tensor.matmul(
                            ps[:],
                            lhsT=ctxT_sb[:, hp, msl],
                            rhs=wo_sb[:, hp, bass.ts(n, SEQT)],
                            start=(hp == 0),
                            stop=(hp == NHP - 1),
                        )
                    ob = outp.tile([128, SEQT], bf16, tag="ob", name="ob")
                    nc.vector.tensor_copy(ob[:], ps[:])
                    nc.sync.dma_start(out[msl, bass.ts(n, SEQT)], ob[:])

            def norm_part1(qb, hp, ctx):
                """DVE-only half of the softmax normalization: pull the
                denominator rows + unnormalized ctx out of PSUM (freeing the
                cx banks) and compute the reciprocals.  No PE instructions,
                so the next head-pair's scores are not blocked behind the
                serial reciprocal chain."""
                dn = small.tile([33, SEQT], f32, tag="dn", name="dn", bufs=2)
                ctmp = [None, None]
                for h in range(2):
                    nc.vector.tensor_copy(
                        dn[32 * h : 32 * h + 1, :], ctx[h][D : D + 1, :]
                    )
                    ctmp[h] = small.tile(
                        [D, SEQT], bf16, tag=f"ctmp{hp}{h}", name="ctmp",
                        bufs=2,
                    )
                    nc.vector.tensor_copy(ctmp[h][:], ctx[h][0:D, :])
                recf = small.tile([33, SEQT], f32, tag=f"recf{hp}", name="recf", bufs=2)
                nc.vector.reciprocal(recf[:], dn[:])
                recr = small.tile([33, SEQT], f32r, tag=f"recr{hp}", name="recr", bufs=2)
                nc.vector.tensor_copy(recr[:], recf[:])
                return ctmp, recr

            def norm_part2(qb, hp, ctmp, recr):
                """PE broadcast of the reciprocals + DVE multiply into ctxT.
                Emitted as a deferred slot closure."""
                qsl = bass.ts(qb, SEQT)
                for h in range(2):
                    p = 32 * h
                    rrep = pp_ps.tile([D, SEQT], f32, tag="pp", name="rrep")
                    nc.tensor.matmul(
                        rrep[:],
                        lhsT=onesr33[p : p + 1, :],
                        rhs=recr[p : p + 1, :],
                        start=True,
                        stop=True,
                    )
                    nc.vector.tensor_tensor(
                        out=ctxT_sb[bass.ds(h * D, D), hp, qsl],
                        in0=ctmp[h][:],
                        in1=rrep[:],
                        op=mybir.AluOpType.mult,
                    )

            def attention(qb, slots):
                """Attention for q-block qb (both head-pairs).  `slots` maps
                slot index (hp*KT + t) -> list of closures emitted between
                that step's exp and ctx matmuls, keeping the PE busy while
                ACT paces the exps.  Returns the norm state for the deferred
                norm_part2."""
                qsl = bass.ts(qb, SEQT)
                norm_state = []

                def emit_scores(hp, t):
                    ksl = bass.ts(t, 128)
                    sc2 = sc_ps.tile([128, 2 * SEQT], f32, tag="sc", name="sc2")
                    for h in range(2):
                        hsl = bass.ts(h, D)
                        nc.tensor.matmul(
                            sc2[:, bass.ts(h, SEQT)],
                            lhsT=kt_sb[hsl, hp, ksl],
                            rhs=qt_sb[hsl, hp, qsl],
                            start=True,
                            stop=True,
                        )
                    return sc2

                for hp in range(NHP):
                    ctx0 = cx_ps.tile([D + 1, SEQT], f32, tag="cx", name="ctx0")
                    ctx1 = cx_ps.tile([D + 1, SEQT], f32, tag="cx", name="ctx1")
                    ctx = (ctx0, ctx1)
                    # Software pipeline: scores(t+1) is emitted BEFORE ctx(t)
                    # so the PE has the next exp's input ready while ctx(t)
                    # waits on exp(t) — ACT runs back-to-back.
                    sc2 = emit_scores(hp, 0)
                    for t in range(KT):
                        pt = ptp.tile([128, 2 * SEQT], bf16, tag="pt", name="pt")
                        nc.scalar.activation(pt[:], sc2[:], Exp, scale=ISD)
                        if t + 1 < KT:
                            sc2 = emit_scores(hp, t + 1)
                        for fn in slots.get(hp * KT + t, ()):
                            fn()
                        for h in range(2):
                            nc.tensor.matmul(
                                ctx[h][:],
                                lhsT=v_sb[:, t, 2 * hp + h, :],
                                rhs=pt[:, bass.ts(h, SEQT)],
                                start=(t == 0),
                                stop=(t == KT - 1),
                            )
                    norm_state.append((qb, hp) + tuple(norm_part1(qb, hp, ctx)))
                return norm_state

            # ---- emission ----
            # Upfront (DMA-paced): K-proj st0/st1 + Q-proj(qb0); everything
            # else rides inside the attention slot schedule.
            kq_proj("kp", wk_sb, xk_sb, bk_sb, kt_sb, 0)
            kq_proj("kp", wk_sb, xk_sb, bk_sb, kt_sb, 1)
            kq_proj("qp", wq_sb, xq_sb, bq_sb, qt_sb, 0)

            def add(slots, i, fn):
                slots.setdefault(i, []).append(fn)

            pending_norm = []
            pending_out = []
            for qb in range(QB):
                slots = {}
                if qb == 0:
                    # V-proj(t) lands right before ctx(t) needs it; the
                    # remaining K-proj tiles arrive before scores reach them.
                    for t in range(SC):
                        add(slots, t, lambda sc=t: v_proj(sc))
                    for hp in range(NHP):
                        add(slots, 2 + 2 * hp, lambda hp=hp: kq_proj_hp(
                            "kp", wk_sb, xk_sb, bk_sb, kt_sb, 2, hp))
                        add(slots, 8 + 2 * hp, lambda hp=hp: kq_proj_hp(
                            "kp", wk_sb, xk_sb, bk_sb, kt_sb, 3, hp))
                    for hp in range(NHP):
                        add(slots, KT + 1 + 2 * hp, lambda hp=hp: kq_proj_hp(
                            "qp", wq_sb, xq_sb, bq_sb, qt_sb, 1, hp))
                else:
                    for i, ns in enumerate(pending_norm):
                        add(slots, 1 + i, lambda ns=ns: norm_part2(*ns))
                    if qb + 1 < QB:
                        for hp in range(NHP):
                            add(slots, 3 + 2 * hp, lambda st=qb + 1, hp=hp:
                                kq_proj_hp("qp", wq_sb, xq_sb, bq_sb,
                                           qt_sb, st, hp))
                    for m, (q, mm) in enumerate(pending_out):
                        add(slots, 8 + 4 * m, lambda q=q, mm=mm: outproj_m(q, mm))
                pending_norm = attention(qb, slots)
                pending_out = [(qb, m) for m in range(4)]
            for ns in pending_norm:
                norm_part2(*ns)
            for q, m in pending_out:
                outproj_m(q, m)

    return nc


def _get_program():
    global _PROGRAM
    if _PROGRAM is None:
        _PROGRAM = _build_program()
    return _PROGRAM


def kernel(query, key, value, Wq, bq, Wk, bk, Wv, bv, Wo, bo):
    from concourse.bass_utils import run_bass_kernel_spmd

    nc = _get_program()
    if not getattr(nc, "_waits_split", False):
        _split_excess_waits(nc)
        nc._waits_split = True

    bf = ml_dtypes.bfloat16
    query = np.asarray(query, np.float32)
    key = np.asarray(key, np.float32)
    value = np.asarray(value, np.float32)
    Wq = np.asarray(Wq, np.float32)
    Wk = np.asarray(Wk, np.float32)
    Wv = np.asarray(Wv, np.float32)
    Wo = np.asarray(Wo, np.float32)
    bq = np.asarray(bq, np.float32)
    bk = np.asarray(bk, np.float32)
    bv = np.asarray(bv, np.float32)
    bo = np.asarray(bo, np.float32)

    # Per-batch x^T [E, S] -> [KE, 128, S] bf16
    xT = {}
    for b in range(B):
        xT[("q", b)] = np.ascontiguousarray(query[b].T).astype(bf).reshape(KE, 128, S)
        xT[("k", b)] = np.ascontiguousarray(key[b].T).astype(bf).reshape(KE, 128, S)
        xT[("v", b)] = np.ascontiguousarray(value[b].T).astype(bf).reshape(KE, 128, S)

    in_maps = []
    for c in range(NCORES):
        b = c // HSPLIT
        g = c % HSPLIT
        rsl = slice(DOUT * g, DOUT * (g + 1))
        in_maps.append(
            {
                "xq": xT[("q", b)], "xk": xT[("k", b)], "xv": xT[("v", b)],
                # lhsT for q/k (and rhs for v): (W_g)^T [E, DOUT]
                "wq": np.ascontiguousarray(Wq[rsl, :].T).astype(bf).reshape(KE, 128, DOUT),
                "wk": np.ascontiguousarray(Wk[rsl, :].T).astype(bf).reshape(KE, 128, DOUT),
                "wv": np.ascontiguousarray(Wv[rsl, :].T).astype(bf).reshape(KE, 128, DOUT),
                # rhs for the out-proj: rows g-range of Wo^T as [NHP, 128, E]
                "wo": np.ascontiguousarray(Wo[:, rsl].T).astype(bf).reshape(NHP, 128, E),
                "bq": np.ascontiguousarray(bq[rsl].reshape(NHP, 128).T),
                "bk": np.ascontiguousarray(bk[rsl].reshape(NHP, 128).T),
                "bv": np.ascontiguousarray(bv[rsl].reshape(1, DOUT)).astype(bf),
            }
        )

    res = run_bass_kernel_spmd(nc, in_maps, list(range(NCORES)), trace=False)
    full = np.empty((B, S, E), np.float32)
    for b in range(B):
        acc = np.zeros((S, E), np.float32)
        for g in range(HSPLIT):
            acc += np.asarray(res.results[b * HSPLIT + g]["out"], np.float32)
        full[b] = acc + bo[None, :]
    return full


# revision 25
# speedup vs baseline: 1.2050x; 1.2050x over previous
"""Multihead attention (B=2, S=2048, E=1024, H=16) on 8 TRN2 cores.

Sharding: 2 batch-groups x 4-way head split.  Core c handles batch c//4
and heads {4g..4g+3} where g = c%4 (DOUT = 256 = 2 head-pairs of 128
partition dims).  Each core computes its partial out-projection in bf16;
the host sums the 4 partials per batch and adds the output bias.

Layout / schedule (per core):
  x^T [E, S] streams in bf16 and persists in SBUF.  K-proj and Q-proj
  produce K^T/Q^T [128 = 2 heads x 64, S] per head-pair (contract E on
  partitions, bias via DVE tensor_scalar_add on the PSUM->SBUF move).
  V-proj runs in the transposed orientation (lhsT = x^T chunk, rhs =
  Wv^T) producing V[s, d] chunks directly -- no PE transposes -- with the
  bias folded in as a rank-1 ones x bv matmul; a DVE copy drops them
  into the [V | ones] ctx lhsT ([128 kpos, 65] per head, ones column
  gives the softmax denominator as ctx row 64).
  Attention per (q-block, head-pair): scores for both heads of the pair
  land in one 2-bank PSUM tile [128, 1024]; ONE Exp activation (scale =
  1/sqrt(D)) covers both, halving ACT instruction overhead -- ACT is the
  pacing engine.  ctx accumulates over the 16 kpos tiles in PSUM.
  Normalization: reciprocal_approx_fast on the denominator row (DVE,
  ~5x faster than the exact reciprocal), PE-broadcast via ones64 lhsT
  (f32r bitcast), one DVE multiply into ctxT (bf16).
  Out-proj (contract the 256 local dims, 2 accumulating matmuls) and
  next q-block's Q-proj are interleaved into the attention t-loop so the
  PE never sits behind a phase barrier.
"""

import numpy as np
import ml_dtypes

# Problem constants (hardcoded per the task contract).
B, S, E, H = 2, 2048, 1024, 16
D = E // H            # 64
NCORES = 8
BGROUPS = 2           # batch groups
HSPLIT = NCORES // BGROUPS   # 4-way head split
DOUT = E // HSPLIT    # 256 local head dims = 4 heads = 2 head-pairs
NHP = DOUT // 128     # 2 head-pairs
KE = E // 128         # 8 contraction tiles over E
SEQT = 512            # seq tile for q/k projections and attention q-blocks
NST = S // SEQT       # 4
QB = S // SEQT        # 4 q-blocks
KT = S // 128         # 16 kpos tiles
SC = S // 128         # 16 seq chunks for the V projection
ISD = float(D) ** -0.5

_PROGRAM = None


# ---------------------------------------------------------------------------
# Workarounds for this walrus build: at most ONE sync wait per instruction is
# reliably accepted ("Too many sync wait commands").  (1) tile's final drain
# gets one wait per logical proc — split them over single-wait SP NOPs;
# (2) a general post-pass moves any instruction's excess waits onto
# preceding same-engine NOPs (engine program order preserves semantics).
# ---------------------------------------------------------------------------


def _install_tile_drain_patch():
    import concourse.mybir as mybir
    import concourse.tile as tile
    from concourse.tile import ScopedClock

    if getattr(tile.TileContext, "_drain_patch_installed", False):
        return

    def _patched_drain_and_barrier(self, tick_clock, wait_clock):
        nc = self.nc
        carrier = nc.sync.nop(nofuse=True)
        wait_clock.add_sem_waits(
            carrier.ins, ScopedClock({None: tick_clock.global_clock})
        )
        si = carrier.ins.sync_info
        waits = list(si.on_wait) if si and si.on_wait else []
        ups = list(si.on_update) if si and si.on_update else []
        if len(waits) > 1:
            carrier.ins.sync_info = mybir.SyncInfo(on_wait=[waits[0]], on_update=ups)
            for w in waits[1:]:
                n2 = nc.sync.nop(nofuse=True)
                n2.ins.sync_info = mybir.SyncInfo(on_wait=[w], on_update=[])
        nc.sync.drain()
        nc.all_engine_barrier()
        popped = nc._tile_sem_poison_stack.pop()
        assert popped is self._sem_poison
        nc.clear_and_free_semaphores(list(self.sems.allocated().values()))
        nc.all_engine_barrier()

    tile.TileContext._drain_and_barrier = _patched_drain_and_barrier
    tile.TileContext._drain_patch_installed = True


MAX_WAITS = 1


def _split_excess_waits(nc):
    import concourse.mybir as mybir

    for bb in nc.main_func.blocks:
        il = list(bb.instructions)
        out = []
        changed = False
        for ins in il:
            si = ins.sync_info
            waits = list(si.on_wait) if si and si.on_wait else []
            if len(waits) > MAX_WAITS:
                changed = True
                extras = waits[: len(waits) - MAX_WAITS]
                keep = waits[len(extras):]
                for i in range(0, len(extras), MAX_WAITS):
                    chunk = extras[i : i + MAX_WAITS]
                    nop = mybir.InstNoOp(
                        name=nc.get_next_instruction_name(), ins=[], outs=[]
                    )
                    nop.engine = ins.engine
                    nop.sync_info = mybir.SyncInfo(on_wait=chunk, on_update=[])
                    out.append(nop)
                ins.sync_info = mybir.SyncInfo(
                    on_wait=keep, on_update=list(si.on_update) if si.on_update else []
                )
            out.append(ins)
        if changed:
            bb.instructions = out


def _build_program():
    import concourse.bass as bass
    import concourse.mybir as mybir
    import concourse.tile as tile

    _install_tile_drain_patch()

    f32 = mybir.dt.float32
    f32r = mybir.dt.float32r
    bf16 = mybir.dt.bfloat16
    Exp = mybir.ActivationFunctionType.Exp

    nc = bass.Bass("TRN2", target_bir_lowering=False, debug=False)

    # DRAM I/O (per core).
    xq = nc.dram_tensor("xq", [KE, 128, S], bf16, kind="ExternalInput").ap()
    xk = nc.dram_tensor("xk", [KE, 128, S], bf16, kind="ExternalInput").ap()
    xv = nc.dram_tensor("xv", [KE, 128, S], bf16, kind="ExternalInput").ap()
    wq = nc.dram_tensor("wq", [KE, 128, DOUT], bf16, kind="ExternalInput").ap()
    wk = nc.dram_tensor("wk", [KE, 128, DOUT], bf16, kind="ExternalInput").ap()
    wv = nc.dram_tensor("wv", [KE, 128, DOUT], bf16, kind="ExternalInput").ap()
    wo = nc.dram_tensor("wo", [NHP, 128, E], bf16, kind="ExternalInput").ap()
    bq = nc.dram_tensor("bq", [128, NHP], f32, kind="ExternalInput").ap()
    bk = nc.dram_tensor("bk", [128, NHP], f32, kind="ExternalInput").ap()
    bv = nc.dram_tensor("bv", [1, DOUT], bf16, kind="ExternalInput").ap()
    out = nc.dram_tensor("out", [S, E], bf16, kind="ExternalOutput").ap()

    with tile.TileContext(nc) as tc:
        with (
            nc.allow_low_precision(reason="bf16 attention pipeline"),
            tc.tile_pool(name="consts", bufs=1) as consts,
            tc.tile_pool(name="persist", bufs=1) as persist,
            tc.tile_pool(name="ptp", bufs=5) as ptp,
            tc.tile_pool(name="outp", bufs=4) as outp,
            tc.tile_pool(name="small", bufs=4) as small,
            tc.tile_pool(name="pp_ps", bufs=2, space="PSUM") as pp_ps,
            tc.tile_pool(name="sc_ps", bufs=2, space="PSUM") as sc_ps,
            tc.tile_pool(name="cx_ps", bufs=2, space="PSUM") as cx_ps,
        ):
            # ---- constants ----
            onesf = consts.tile([128, 1], f32)
            nc.vector.memset(onesf[:], 1.0)
            # ones rows (at base partitions 0 and 32) for the reciprocal
            # broadcast matmuls; f32r via copy (memset can't write f32r)
            onesr33 = consts.tile([33, 64], f32r)
            nc.vector.tensor_copy(
                onesr33[:], onesf[0:33, 0:1].broadcast_to([33, 64])
            )
            ones1b = consts.tile([1, 128], bf16)
            nc.vector.memset(ones1b[:], 1.0)

            # ---- persistent weights ----
            wq_sb = persist.tile([128, KE, DOUT], bf16, tag="wq")
            wk_sb = persist.tile([128, KE, DOUT], bf16, tag="wk")
            wv_sb = persist.tile([128, KE, DOUT], bf16, tag="wv")
            wo_sb = persist.tile([128, NHP, E], bf16, tag="wo")
            bq_sb = persist.tile([128, NHP], f32, tag="bq")
            bk_sb = persist.tile([128, NHP], f32, tag="bk")
            bv_sb = persist.tile([1, DOUT], bf16, tag="bv")

            # ---- persistent activations ----
            xq_sb = persist.tile([128, KE, S], bf16, tag="xq")
            xk_sb = persist.tile([128, KE, S], bf16, tag="xk")
            xv_sb = persist.tile([128, KE, S], bf16, tag="xv")
            qt_sb = persist.tile([128, NHP, S], bf16, tag="qt")
            kt_sb = persist.tile([128, NHP, S], bf16, tag="kt")
            # [V | ones] per (kpos chunk, local head): ctx lhsT
            v_sb = persist.tile([128, SC, 4, D + 1], bf16, tag="vn")
            ctxT_sb = persist.tile([128, NHP, S], bf16, tag="ctxT")

            nc.vector.tensor_copy(
                v_sb[:, :, :, D], onesf[:, 0:1].broadcast_to([128, SC, 4])
            )

            # ---- input DMAs ----
            # Trigger dispatch costs ~640ns per dma_start on an engine's
            # queue, so the ~125 triggers are spread across three engines
            # (sync / vector / gpsimd) in first-use order; x tensors are
            # chunked [128, 512] so transfers parallelize across the 16 DMA
            # queues and consumers start on partial data.
            sl0 = bass.ts(0, SEQT)
            # sync: the K path (interleave weight/x per k so K-proj can
            # start contracting as chunks land), then the later K tiles
            for k in range(KE):
                nc.sync.dma_start(wk_sb[:, k, :], wk[k])
                nc.sync.dma_start(xk_sb[:, k, sl0], xk[k, :, sl0])
            nc.sync.dma_start(bk_sb[:], bk[:])
            nc.sync.dma_start(bq_sb[:], bq[:])
            for st in range(1, NST):
                sl = bass.ts(st, SEQT)
                for k in range(KE):
                    nc.sync.dma_start(xk_sb[:, k, sl], xk[k, :, sl])
            # gpsimd: the Q path, then V chunks and late Q tiles
            for k in range(KE):
                nc.gpsimd.dma_start(wq_sb[:, k, :], wq[k])
                nc.gpsimd.dma_start(xq_sb[:, k, sl0], xq[k, :, sl0])
            for scg in range(2):
                sl = bass.ts(scg, SEQT)
                for k in range(KE):
                    nc.gpsimd.dma_start(xv_sb[:, k, sl], xv[k, :, sl])
            for k in range(KE):
                sl = bass.ts(1, SEQT)
                nc.gpsimd.dma_start(xq_sb[:, k, sl], xq[k, :, sl])
            for scg in range(2, NST):
                sl = bass.ts(scg, SEQT)
                for k in range(KE):
                    nc.gpsimd.dma_start(xv_sb[:, k, sl], xv[k, :, sl])
            for st in range(2, NST):
                sl = bass.ts(st, SEQT)
                for k in range(KE):
                    nc.gpsimd.dma_start(xq_sb[:, k, sl], xq[k, :, sl])
            for hp in range(NHP):
                nc.gpsimd.dma_start(wo_sb[:, hp, :], wo[hp])
            # scalar: the V weights (ACT is otherwise idle until the first
            # exp; its DMA dispatch is slow (~1.4us each) so it gets few)
            for k in range(KE):
                nc.scalar.dma_start(wv_sb[:, k, :], wv[k])
            nc.scalar.dma_start(bv_sb[:], bv[:])

            def kq_proj_hp(name, w_sb, x_sb, b_sb, dst, st, hp):
                """One [128, 512] tile of the K or Q projection (transposed
                output layout [d, s]) for one head-pair + bias add."""
                sl = bass.ts(st, SEQT)
                ps = pp_ps.tile([128, SEQT], f32, tag="pp", name=f"{name}{st}{hp}")
                for k in range(KE):
                    nc.tensor.matmul(
                        ps[:],
                        lhsT=w_sb[:, k, bass.ts(hp, 128)],
                        rhs=x_sb[:, k, sl],
                        start=(k == 0),
                        stop=(k == KE - 1),
                    )
                nc.vector.tensor_scalar_add(
                    dst[:, hp, sl], ps[:], b_sb[:, hp : hp + 1]
                )

            def kq_proj(name, w_sb, x_sb, b_sb, dst, st):
                for hp in range(NHP):
                    kq_proj_hp(name, w_sb, x_sb, b_sb, dst, st, hp)

            def v_proj(sc):
                """V rows [128 seq, 256 d] directly via lhsT = x^T chunk;
                bias folded in as a rank-1 (ones x bv) accumulate."""
                ssl = bass.ts(sc, 128)
                ps = pp_ps.tile([128, SEQT], f32, tag="pp", name=f"vp{sc}")
                vp = ps[:, 0:DOUT]
                for k in range(KE):
                    nc.tensor.matmul(
                        vp,
                        lhsT=xv_sb[:, k, ssl],
                        rhs=wv_sb[:, k, :],
                        start=(k == 0),
                        stop=False,
                    )
                nc.tensor.matmul(
                    vp, lhsT=ones1b[:], rhs=bv_sb[:], start=False, stop=True
                )
                for h in range(4):
                    nc.vector.tensor_copy(
                        v_sb[:, sc, h, 0:D], ps[:, bass.ts(h, D)]
                    )

            def outproj_m(qb, m):
                """One output row-tile [128 seq, E]: contract the 256 local
                dims (2 accumulating matmuls per 512-wide chunk), copy to
                SBUF bf16, DMA out."""
                msl = bass.ts(4 * qb + m, 128)
                for n in range(E // SEQT):
                    ps = pp_ps.tile([128, SEQT], f32, tag="pp", name=f"ob{qb}{m}{n}")
                    for hp in range(NHP):
                        nc.tensor.matmul(
                            ps[:],
                            lhsT=ctxT_sb[:, hp, msl],
                            rhs=wo_sb[:, hp, bass.ts(n, SEQT)],
                            start=(hp == 0),
                            stop=(hp == NHP - 1),
                        )
                    ob = outp.tile([128, SEQT], bf16, tag="ob", name="ob")
                    nc.vector.tensor_copy(ob[:], ps[:])
                    nc.sync.dma_start(out[msl, bass.ts(n, SEQT)], ob[:])

            def norm_part1(hp, ctx):
                """DVE-only half of the softmax normalization: pull the
                unnormalized ctx (with the denominator row, bf16) out of
                PSUM — freeing the cx banks after just two casts — then
                gather denominators and compute the reciprocals.  No PE
                instructions, so the scores->exp chain never blocks on the
                serial reciprocal."""
                ctmp = [None, None]
                for h in range(2):
                    ctmp[h] = small.tile(
                        [D + 1, SEQT], bf16, tag=f"ctmp{hp}{h}", name="ctmp",
                        bufs=2,
                    )
                    nc.vector.tensor_copy(ctmp[h][:], ctx[h][:])
                dn = small.tile([33, SEQT], f32, tag="dn", name="dn", bufs=2)
                for h in range(2):
                    nc.vector.tensor_copy(
                        dn[32 * h : 32 * h + 1, :], ctmp[h][D : D + 1, :]
                    )
                recf = small.tile([33, SEQT], f32, tag=f"recf{hp}", name="recf", bufs=2)
                nc.vector.reciprocal(recf[:], dn[:])
                recr = small.tile([33, SEQT], f32r, tag=f"recr{hp}", name="recr", bufs=2)
                nc.vector.tensor_copy(recr[:], recf[:])
                return ctmp, recr

            def norm_part2(qb, hp, ctmp, recr):
                """PE broadcast of the reciprocals + DVE multiply into ctxT.
                Emitted as a deferred slot closure."""
                qsl = bass.ts(qb, SEQT)
                for h in range(2):
                    p = 32 * h
                    rrep = pp_ps.tile([D, SEQT], f32, tag="pp", name="rrep")
                    nc.tensor.matmul(
                        rrep[:],
                        lhsT=onesr33[p : p + 1, :],
                        rhs=recr[p : p + 1, :],
                        start=True,
                        stop=True,
                    )
                    nc.vector.tensor_tensor(
                        out=ctxT_sb[bass.ds(h * D, D), hp, qsl],
                        in0=ctmp[h][0:D, :],
                        in1=rrep[:],
                        op=mybir.AluOpType.mult,
                    )

            # ---- emission ----
            # Upfront (DMA-paced): K-proj st0/st1 + Q-proj(qb0).
            kq_proj("kp", wk_sb, xk_sb, bk_sb, kt_sb, 0)
            kq_proj("kp", wk_sb, xk_sb, bk_sb, kt_sb, 1)
            kq_proj("qp", wq_sb, xq_sb, bq_sb, qt_sb, 0)

            # Attention as ONE flat stream of NBLK x KT steps (block =
            # (q-block, head-pair)).  Step i: exp(i) -> scores(i+1) ->
            # slot closures -> ctx(i - LAG).  The 2-step ctx lag keeps every
            # data wait (V tiles, cx frees, norm chains) off the
            # scores->exp critical chain; ACT paces the whole kernel.
            NBLK = QB * NHP
            NSTEP = NBLK * KT
            LAG = 2
            slots = {}

            def add(i, fn):
                slots.setdefault(i, []).append(fn)

            def emit_scores(i):
                b, t = divmod(i, KT)
                qb, hp = divmod(b, NHP)
                sc2 = sc_ps.tile([128, 2 * SEQT], f32, tag="sc", name="sc2")
                for h in range(2):
                    hsl = bass.ts(h, D)
                    nc.tensor.matmul(
                        sc2[:, bass.ts(h, SEQT)],
                        lhsT=kt_sb[hsl, hp, bass.ts(t, 128)],
                        rhs=qt_sb[hsl, hp, bass.ts(qb, SEQT)],
                        start=True,
                        stop=True,
                    )
                return sc2

            # static slot schedule
            for t in range(SC):              # build v_sb during block 0/1
                add(t, lambda sc=t: v_proj(sc))
            for hp in range(NHP):            # remaining K tiles
                add(5 + hp, lambda hp=hp: kq_proj_hp(
                    "kp", wk_sb, xk_sb, bk_sb, kt_sb, 2, hp))
                add(9 + hp, lambda hp=hp: kq_proj_hp(
                    "kp", wk_sb, xk_sb, bk_sb, kt_sb, 3, hp))
            for qb in range(QB - 1):         # next q-block's Q-proj
                for hp in range(NHP):
                    add(32 * qb + 24 + 2 * hp, lambda st=qb + 1, hp=hp:
                        kq_proj_hp("qp", wq_sb, xq_sb, bq_sb, qt_sb, st, hp))
            for qb in range(QB - 1):         # out-proj of the previous qb
                for m in range(4):
                    add(32 * (qb + 1) + 10 + 4 * m,
                        lambda q=qb, mm=m: outproj_m(q, mm))

            norm_info = {}
            ctx_cur = None
            pts = {}
            sc_cur = emit_scores(0)
            for i in range(NSTEP + LAG + 1):
                if i < NSTEP:
                    pt = ptp.tile([128, 2 * SEQT], bf16, tag="pt", name="pt")
                    nc.scalar.activation(pt[:], sc_cur[:], Exp, scale=ISD)
                    pts[i] = pt
                if i + 1 < NSTEP:
                    sc_cur = emit_scores(i + 1)
                for fn in slots.pop(i, ()):
                    fn()
                j = i - LAG
                if 0 <= j < NSTEP:
                    bj, tj = divmod(j, KT)
                    qbj, hpj = divmod(bj, NHP)
                    if tj == 0:
                        ctx_cur = (
                            cx_ps.tile([D + 1, SEQT], f32, tag="cx", name="c0"),
                            cx_ps.tile([D + 1, SEQT], f32, tag="cx", name="c1"),
                        )
                    ptj = pts.pop(j)
                    for h in range(2):
                        nc.tensor.matmul(
                            ctx_cur[h][:],
                            lhsT=v_sb[:, tj, 2 * hpj + h, :],
                            rhs=ptj[:, bass.ts(h, SEQT)],
                            start=(tj == 0),
                            stop=(tj == KT - 1),
                        )
                    if tj == KT - 1:
                        def make_part1(bb, cc):
                            def run():
                                qbb, hpb = divmod(bb, NHP)
                                norm_info[bb] = (qbb, hpb) + tuple(
                                    norm_part1(hpb, cc)
                                )
                            return run
                        add(i + 1, make_part1(bj, ctx_cur))
                        def make_part2(bb):
                            def run():
                                norm_part2(*norm_info.pop(bb))
                            return run
                        add(16 * bj + 24, make_part2(bj))
            # tail: block 7's norm + the last q-block's out-proj
            for i in sorted(slots):
                for fn in slots.pop(i, ()):
                    fn()
            for m in range(4):
                outproj_m(QB - 1, m)

    return nc


def _get_program():
    global _PROGRAM
    if _PROGRAM is None:
        _PROGRAM = _build_program()
    return _PROGRAM


def kernel(query, key, value, Wq, bq, Wk, bk, Wv, bv, Wo, bo):
    from concourse.bass_utils import run_bass_kernel_spmd

    nc = _get_program()
    if not getattr(nc, "_waits_split", False):
        _split_excess_waits(nc)
        nc._waits_split = True

    bf = ml_dtypes.bfloat16
    query = np.asarray(query, np.float32)
    key = np.asarray(key, np.float32)
    value = np.asarray(value, np.float32)
    Wq = np.asarray(Wq, np.float32)
    Wk = np.asarray(Wk, np.float32)
    Wv = np.asarray(Wv, np.float32)
    Wo = np.asarray(Wo, np.float32)
    bq = np.asarray(bq, np.float32)
    bk = np.asarray(bk, np.float32)
    bv = np.asarray(bv, np.float32)
    bo = np.asarray(bo, np.float32)

    # Per-batch x^T [E, S] -> [KE, 128, S] bf16
    xT = {}
    for b in range(B):
        xT[("q", b)] = np.ascontiguousarray(query[b].T).astype(bf).reshape(KE, 128, S)
        xT[("k", b)] = np.ascontiguousarray(key[b].T).astype(bf).reshape(KE, 128, S)
        xT[("v", b)] = np.ascontiguousarray(value[b].T).astype(bf).reshape(KE, 128, S)

    in_maps = []
    for c in range(NCORES):
        b = c // HSPLIT
        g = c % HSPLIT
        rsl = slice(DOUT * g, DOUT * (g + 1))
        in_maps.append(
            {
                "xq": xT[("q", b)], "xk": xT[("k", b)], "xv": xT[("v", b)],
                # lhsT for q/k (and rhs for v): (W_g)^T [E, DOUT]
                "wq": np.ascontiguousarray(Wq[rsl, :].T).astype(bf).reshape(KE, 128, DOUT),
                "wk": np.ascontiguousarray(Wk[rsl, :].T).astype(bf).reshape(KE, 128, DOUT),
                "wv": np.ascontiguousarray(Wv[rsl, :].T).astype(bf).reshape(KE, 128, DOUT),
                # rhs for the out-proj: rows g-range of Wo^T as [NHP, 128, E]
                "wo": np.ascontiguousarray(Wo[:, rsl].T).astype(bf).reshape(NHP, 128, E),
                "bq": np.ascontiguousarray(bq[rsl].reshape(NHP, 128).T),
                "bk": np.ascontiguousarray(bk[rsl].reshape(NHP, 128).T),
                "bv": np.ascontiguousarray(bv[rsl].reshape(1, DOUT)).astype(bf),
            }
        )

    res = run_bass_kernel_spmd(nc, in_maps, list(range(NCORES)), trace=False)
    full = np.empty((B, S, E), np.float32)
    for b in range(B):
        acc = np.zeros((S, E), np.float32)
        for g in range(HSPLIT):
            acc += np.asarray(res.results[b * HSPLIT + g]["out"], np.float32)
        full[b] = acc + bo[None, :]
    return full


# revision 26
# speedup vs baseline: 1.2635x; 1.0486x over previous
"""Multihead attention (B=2, S=2048, E=1024, H=16) on 8 TRN2 cores.

Sharding: 2 batch-groups x 4-way head split.  Core c handles batch c//4
and heads {4g..4g+3} where g = c%4 (DOUT = 256 = 2 head-pairs of 128
partition dims).  Each core computes its partial out-projection in bf16;
the host sums the 4 partials per batch and adds the output bias.

Layout / schedule (per core):
  x^T [E, S] streams in bf16 and persists in SBUF.  K-proj and Q-proj
  produce K^T/Q^T [128 = 2 heads x 64, S] per head-pair (contract E on
  partitions, bias via DVE tensor_scalar_add on the PSUM->SBUF move).
  V-proj runs in the transposed orientation (lhsT = x^T chunk, rhs =
  Wv^T) producing V[s, d] chunks directly -- no PE transposes -- with the
  bias folded in as a rank-1 ones x bv matmul; a DVE copy drops them
  into the [V | ones] ctx lhsT ([128 kpos, 65] per head, ones column
  gives the softmax denominator as ctx row 64).
  Attention per (q-block, head-pair): scores for both heads of the pair
  land in one 2-bank PSUM tile [128, 1024]; ONE Exp activation (scale =
  1/sqrt(D)) covers both, halving ACT instruction overhead -- ACT is the
  pacing engine.  ctx accumulates over the 16 kpos tiles in PSUM.
  Normalization: reciprocal_approx_fast on the denominator row (DVE,
  ~5x faster than the exact reciprocal), PE-broadcast via ones64 lhsT
  (f32r bitcast), one DVE multiply into ctxT (bf16).
  Out-proj (contract the 256 local dims, 2 accumulating matmuls) and
  next q-block's Q-proj are interleaved into the attention t-loop so the
  PE never sits behind a phase barrier.
"""

import numpy as np
import ml_dtypes

# Problem constants (hardcoded per the task contract).
B, S, E, H = 2, 2048, 1024, 16
D = E // H            # 64
NCORES = 8
BGROUPS = 2           # batch groups
HSPLIT = NCORES // BGROUPS   # 4-way head split
DOUT = E // HSPLIT    # 256 local head dims = 4 heads = 2 head-pairs
NHP = DOUT // 128     # 2 head-pairs
KE = E // 128         # 8 contraction tiles over E
SEQT = 512            # seq tile for q/k projections and attention q-blocks
NST = S // SEQT       # 4
QB = S // SEQT        # 4 q-blocks
KT = S // 128         # 16 kpos tiles
SC = S // 128         # 16 seq chunks for the V projection
ISD = float(D) ** -0.5

_PROGRAM = None


# ---------------------------------------------------------------------------
# Workarounds for this walrus build: at most ONE sync wait per instruction is
# reliably accepted ("Too many sync wait commands").  (1) tile's final drain
# gets one wait per logical proc — split them over single-wait SP NOPs;
# (2) a general post-pass moves any instruction's excess waits onto
# preceding same-engine NOPs (engine program order preserves semantics).
# ---------------------------------------------------------------------------


def _install_tile_drain_patch():
    import concourse.mybir as mybir
    import concourse.tile as tile
    from concourse.tile import ScopedClock

    if getattr(tile.TileContext, "_drain_patch_installed", False):
        return

    def _patched_drain_and_barrier(self, tick_clock, wait_clock):
        nc = self.nc
        carrier = nc.sync.nop(nofuse=True)
        wait_clock.add_sem_waits(
            carrier.ins, ScopedClock({None: tick_clock.global_clock})
        )
        si = carrier.ins.sync_info
        waits = list(si.on_wait) if si and si.on_wait else []
        ups = list(si.on_update) if si and si.on_update else []
        if len(waits) > 1:
            carrier.ins.sync_info = mybir.SyncInfo(on_wait=[waits[0]], on_update=ups)
            for w in waits[1:]:
                n2 = nc.sync.nop(nofuse=True)
                n2.ins.sync_info = mybir.SyncInfo(on_wait=[w], on_update=[])
        nc.sync.drain()
        nc.all_engine_barrier()
        popped = nc._tile_sem_poison_stack.pop()
        assert popped is self._sem_poison
        nc.clear_and_free_semaphores(list(self.sems.allocated().values()))
        nc.all_engine_barrier()

    tile.TileContext._drain_and_barrier = _patched_drain_and_barrier
    tile.TileContext._drain_patch_installed = True


MAX_WAITS = 1


def _split_excess_waits(nc):
    import concourse.mybir as mybir

    for bb in nc.main_func.blocks:
        il = list(bb.instructions)
        out = []
        changed = False
        for ins in il:
            si = ins.sync_info
            waits = list(si.on_wait) if si and si.on_wait else []
            if len(waits) > MAX_WAITS:
                changed = True
                extras = waits[: len(waits) - MAX_WAITS]
                keep = waits[len(extras):]
                for i in range(0, len(extras), MAX_WAITS):
                    chunk = extras[i : i + MAX_WAITS]
                    nop = mybir.InstNoOp(
                        name=nc.get_next_instruction_name(), ins=[], outs=[]
                    )
                    nop.engine = ins.engine
                    nop.sync_info = mybir.SyncInfo(on_wait=chunk, on_update=[])
                    out.append(nop)
                ins.sync_info = mybir.SyncInfo(
                    on_wait=keep, on_update=list(si.on_update) if si.on_update else []
                )
            out.append(ins)
        if changed:
            bb.instructions = out


def _build_program():
    import concourse.bass as bass
    import concourse.mybir as mybir
    import concourse.tile as tile

    _install_tile_drain_patch()

    f32 = mybir.dt.float32
    f32r = mybir.dt.float32r
    bf16 = mybir.dt.bfloat16
    Exp = mybir.ActivationFunctionType.Exp

    nc = bass.Bass("TRN2", target_bir_lowering=False, debug=False)

    # DRAM I/O (per core).
    xq = nc.dram_tensor("xq", [KE, 128, S], bf16, kind="ExternalInput").ap()
    xk = nc.dram_tensor("xk", [KE, 128, S], bf16, kind="ExternalInput").ap()
    xv = nc.dram_tensor("xv", [KE, 128, S], bf16, kind="ExternalInput").ap()
    wq = nc.dram_tensor("wq", [KE, 128, DOUT], bf16, kind="ExternalInput").ap()
    wk = nc.dram_tensor("wk", [KE, 128, DOUT], bf16, kind="ExternalInput").ap()
    wv = nc.dram_tensor("wv", [KE, 128, DOUT], bf16, kind="ExternalInput").ap()
    wo = nc.dram_tensor("wo", [NHP, 128, E], bf16, kind="ExternalInput").ap()
    bq = nc.dram_tensor("bq", [128, NHP], f32, kind="ExternalInput").ap()
    bk = nc.dram_tensor("bk", [128, NHP], f32, kind="ExternalInput").ap()
    bv = nc.dram_tensor("bv", [1, DOUT], bf16, kind="ExternalInput").ap()
    out = nc.dram_tensor("out", [S, E], bf16, kind="ExternalOutput").ap()

    with tile.TileContext(nc) as tc:
        with (
            nc.allow_low_precision(reason="bf16 attention pipeline"),
            tc.tile_pool(name="consts", bufs=1) as consts,
            tc.tile_pool(name="persist", bufs=1) as persist,
            tc.tile_pool(name="ptp", bufs=5) as ptp,
            tc.tile_pool(name="outp", bufs=4) as outp,
            tc.tile_pool(name="small", bufs=4) as small,
            tc.tile_pool(name="pp_ps", bufs=2, space="PSUM") as pp_ps,
            tc.tile_pool(name="sc_ps", bufs=2, space="PSUM") as sc_ps,
            tc.tile_pool(name="cx_ps", bufs=2, space="PSUM") as cx_ps,
        ):
            # ---- constants ----
            onesf = consts.tile([128, 1], f32)
            nc.vector.memset(onesf[:], 1.0)
            # ones rows (at base partitions 0 and 32) for the reciprocal
            # broadcast matmuls; f32r via copy (memset can't write f32r)
            onesr33 = consts.tile([33, 64], f32r)
            nc.vector.tensor_copy(
                onesr33[:], onesf[0:33, 0:1].broadcast_to([33, 64])
            )
            ones1b = consts.tile([1, 128], bf16)
            nc.vector.memset(ones1b[:], 1.0)

            # ---- persistent weights ----
            wq_sb = persist.tile([128, KE, DOUT], bf16, tag="wq")
            wk_sb = persist.tile([128, KE, DOUT], bf16, tag="wk")
            wv_sb = persist.tile([128, KE, DOUT], bf16, tag="wv")
            wo_sb = persist.tile([128, NHP, E], bf16, tag="wo")
            bq_sb = persist.tile([128, NHP], f32, tag="bq")
            bk_sb = persist.tile([128, NHP], f32, tag="bk")
            bv_sb = persist.tile([1, DOUT], bf16, tag="bv")

            # ---- persistent activations ----
            xq_sb = persist.tile([128, KE, S], bf16, tag="xq")
            xk_sb = persist.tile([128, KE, S], bf16, tag="xk")
            xv_sb = persist.tile([128, KE, S], bf16, tag="xv")
            qt_sb = persist.tile([128, NHP, S], bf16, tag="qt")
            kt_sb = persist.tile([128, NHP, S], bf16, tag="kt")
            # [V | ones] per (kpos chunk, local head): ctx lhsT
            v_sb = persist.tile([128, SC, 4, D + 1], bf16, tag="vn")
            ctxT_sb = persist.tile([128, NHP, S], bf16, tag="ctxT")

            nc.vector.tensor_copy(
                v_sb[:, :, :, D], onesf[:, 0:1].broadcast_to([128, SC, 4])
            )

            # ---- input DMAs ----
            # Trigger dispatch costs ~640ns per dma_start on an engine's
            # queue, so the ~125 triggers are spread across three engines
            # (sync / vector / gpsimd) in first-use order; x tensors are
            # chunked [128, 512] so transfers parallelize across the 16 DMA
            # queues and consumers start on partial data.
            sl0 = bass.ts(0, SEQT)
            # sync: the K path (interleave weight/x per k so K-proj can
            # start contracting as chunks land), then the later K tiles
            for k in range(KE):
                nc.sync.dma_start(wk_sb[:, k, :], wk[k])
                nc.sync.dma_start(xk_sb[:, k, sl0], xk[k, :, sl0])
            nc.sync.dma_start(bk_sb[:], bk[:])
            nc.sync.dma_start(bq_sb[:], bq[:])
            for st in range(1, NST):
                sl = bass.ts(st, SEQT)
                for k in range(KE):
                    nc.sync.dma_start(xk_sb[:, k, sl], xk[k, :, sl])
            # gpsimd: the Q path, then V chunks and late Q tiles
            for k in range(KE):
                nc.gpsimd.dma_start(wq_sb[:, k, :], wq[k])
                nc.gpsimd.dma_start(xq_sb[:, k, sl0], xq[k, :, sl0])
            for scg in range(2):
                sl = bass.ts(scg, SEQT)
                for k in range(KE):
                    nc.gpsimd.dma_start(xv_sb[:, k, sl], xv[k, :, sl])
            for k in range(KE):
                sl = bass.ts(1, SEQT)
                nc.gpsimd.dma_start(xq_sb[:, k, sl], xq[k, :, sl])
            for scg in range(2, NST):
                sl = bass.ts(scg, SEQT)
                for k in range(KE):
                    nc.gpsimd.dma_start(xv_sb[:, k, sl], xv[k, :, sl])
            for st in range(2, NST):
                sl = bass.ts(st, SEQT)
                for k in range(KE):
                    nc.gpsimd.dma_start(xq_sb[:, k, sl], xq[k, :, sl])
            for hp in range(NHP):
                nc.gpsimd.dma_start(wo_sb[:, hp, :], wo[hp])
            # scalar: the V weights (ACT is otherwise idle until the first
            # exp; its DMA dispatch is slow (~1.4us each) so it gets few)
            for k in range(KE):
                nc.scalar.dma_start(wv_sb[:, k, :], wv[k])
            nc.scalar.dma_start(bv_sb[:], bv[:])

            def kq_proj_hp(name, w_sb, x_sb, b_sb, dst, st, hp):
                """One [128, 512] tile of the K or Q projection (transposed
                output layout [d, s]) for one head-pair + bias add."""
                sl = bass.ts(st, SEQT)
                ps = pp_ps.tile([128, SEQT], f32, tag="pp", name=f"{name}{st}{hp}")
                for k in range(KE):
                    nc.tensor.matmul(
                        ps[:],
                        lhsT=w_sb[:, k, bass.ts(hp, 128)],
                        rhs=x_sb[:, k, sl],
                        start=(k == 0),
                        stop=(k == KE - 1),
                    )
                nc.vector.tensor_scalar_add(
                    dst[:, hp, sl], ps[:], b_sb[:, hp : hp + 1]
                )

            def kq_proj(name, w_sb, x_sb, b_sb, dst, st):
                for hp in range(NHP):
                    kq_proj_hp(name, w_sb, x_sb, b_sb, dst, st, hp)

            def v_proj(sc):
                """V rows [128 seq, 256 d] directly via lhsT = x^T chunk;
                bias folded in as a rank-1 (ones x bv) accumulate."""
                ssl = bass.ts(sc, 128)
                ps = pp_ps.tile([128, SEQT], f32, tag="pp", name=f"vp{sc}")
                vp = ps[:, 0:DOUT]
                for k in range(KE):
                    nc.tensor.matmul(
                        vp,
                        lhsT=xv_sb[:, k, ssl],
                        rhs=wv_sb[:, k, :],
                        start=(k == 0),
                        stop=False,
                    )
                nc.tensor.matmul(
                    vp, lhsT=ones1b[:], rhs=bv_sb[:], start=False, stop=True
                )
                for h in range(4):
                    nc.vector.tensor_copy(
                        v_sb[:, sc, h, 0:D], ps[:, bass.ts(h, D)]
                    )

            def outproj_m(qb, m):
                """One output row-tile [128 seq, E]: contract the 256 local
                dims (2 accumulating matmuls per 512-wide chunk), copy to
                SBUF bf16, DMA out."""
                msl = bass.ts(4 * qb + m, 128)
                for n in range(E // SEQT):
                    ps = pp_ps.tile([128, SEQT], f32, tag="pp", name=f"ob{qb}{m}{n}")
                    for hp in range(NHP):
                        nc.tensor.matmul(
                            ps[:],
                            lhsT=ctxT_sb[:, hp, msl],
                            rhs=wo_sb[:, hp, bass.ts(n, SEQT)],
                            start=(hp == 0),
                            stop=(hp == NHP - 1),
                        )
                    ob = outp.tile([128, SEQT], bf16, tag="ob", name="ob")
                    nc.vector.tensor_copy(ob[:], ps[:])
                    nc.gpsimd.dma_start(out[msl, bass.ts(n, SEQT)], ob[:])

            def norm_part1(hp, ctx):
                """DVE-only half of the softmax normalization: pull the
                unnormalized ctx (with the denominator row, bf16) out of
                PSUM — freeing the cx banks after just two casts — then
                gather denominators and compute the reciprocals.  No PE
                instructions, so the scores->exp chain never blocks on the
                serial reciprocal."""
                ctmp = [None, None]
                for h in range(2):
                    ctmp[h] = small.tile(
                        [D + 1, SEQT], bf16, tag=f"ctmp{hp}{h}", name="ctmp",
                        bufs=2,
                    )
                    nc.vector.tensor_copy(ctmp[h][:], ctx[h][:])
                dn = small.tile([33, SEQT], f32, tag="dn", name="dn", bufs=2)
                for h in range(2):
                    nc.vector.tensor_copy(
                        dn[32 * h : 32 * h + 1, :], ctmp[h][D : D + 1, :]
                    )
                recf = small.tile([33, SEQT], f32, tag=f"recf{hp}", name="recf", bufs=2)
                nc.vector.reciprocal(recf[:], dn[:])
                recr = small.tile([33, SEQT], f32r, tag=f"recr{hp}", name="recr", bufs=2)
                nc.vector.tensor_copy(recr[:], recf[:])
                return ctmp, recr

            def norm_part2(qb, hp, ctmp, recr):
                """PE broadcast of the reciprocals + DVE multiply into ctxT.
                Emitted as a deferred slot closure."""
                qsl = bass.ts(qb, SEQT)
                for h in range(2):
                    p = 32 * h
                    rrep = pp_ps.tile([D, SEQT], f32, tag="pp", name="rrep")
                    nc.tensor.matmul(
                        rrep[:],
                        lhsT=onesr33[p : p + 1, :],
                        rhs=recr[p : p + 1, :],
                        start=True,
                        stop=True,
                    )
                    nc.vector.tensor_tensor(
                        out=ctxT_sb[bass.ds(h * D, D), hp, qsl],
                        in0=ctmp[h][0:D, :],
                        in1=rrep[:],
                        op=mybir.AluOpType.mult,
                    )

            # ---- emission ----
            # Upfront (DMA-paced): K-proj st0/st1 + Q-proj(qb0).
            kq_proj("kp", wk_sb, xk_sb, bk_sb, kt_sb, 0)
            kq_proj("qp", wq_sb, xq_sb, bq_sb, qt_sb, 0)

            # Attention as ONE flat stream of NBLK x KT steps (block =
            # (q-block, head-pair)).  Step i: exp(i) -> scores(i+1) ->
            # slot closures -> ctx(i - LAG).  The 2-step ctx lag keeps every
            # data wait (V tiles, cx frees, norm chains) off the
            # scores->exp critical chain; ACT paces the whole kernel.
            NBLK = QB * NHP
            NSTEP = NBLK * KT
            LAG = 2
            slots = {}

            def add(i, fn):
                slots.setdefault(i, []).append(fn)

            def emit_scores(i):
                b, t = divmod(i, KT)
                qb, hp = divmod(b, NHP)
                sc2 = sc_ps.tile([128, 2 * SEQT], f32, tag="sc", name="sc2")
                for h in range(2):
                    hsl = bass.ts(h, D)
                    nc.tensor.matmul(
                        sc2[:, bass.ts(h, SEQT)],
                        lhsT=kt_sb[hsl, hp, bass.ts(t, 128)],
                        rhs=qt_sb[hsl, hp, bass.ts(qb, SEQT)],
                        start=True,
                        stop=True,
                    )
                return sc2

            # static slot schedule
            for t in range(SC):              # build v_sb during block 0/1
                add(t + 1, lambda sc=t: v_proj(sc))
            for hp in range(NHP):            # remaining K tiles; each must
                add(1 + hp, lambda hp=hp: kq_proj_hp(   # land before the
                    "kp", wk_sb, xk_sb, bk_sb, kt_sb, 1, hp))  # scores that
                add(5 + hp, lambda hp=hp: kq_proj_hp(   # read it (emitted
                    "kp", wk_sb, xk_sb, bk_sb, kt_sb, 2, hp))  # at step 4t-1)
                add(9 + hp, lambda hp=hp: kq_proj_hp(
                    "kp", wk_sb, xk_sb, bk_sb, kt_sb, 3, hp))
            for qb in range(QB - 1):         # next q-block's Q-proj
                for hp in range(NHP):
                    add(32 * qb + 24 + 2 * hp, lambda st=qb + 1, hp=hp:
                        kq_proj_hp("qp", wq_sb, xq_sb, bq_sb, qt_sb, st, hp))
            for qb in range(QB - 1):         # out-proj of the previous qb
                for m in range(4):
                    add(32 * (qb + 1) + 16 + 4 * m,
                        lambda q=qb, mm=m: outproj_m(q, mm))

            norm_info = {}
            ctx_cur = None
            pts = {}
            sc_cur = emit_scores(0)
            for i in range(NSTEP + LAG + 1):
                if i < NSTEP:
                    pt = ptp.tile([128, 2 * SEQT], bf16, tag="pt", name="pt")
                    nc.scalar.activation(pt[:], sc_cur[:], Exp, scale=ISD)
                    pts[i] = pt
                if i + 1 < NSTEP:
                    sc_cur = emit_scores(i + 1)
                for fn in slots.pop(i, ()):
                    fn()
                j = i - LAG
                if 0 <= j < NSTEP:
                    bj, tj = divmod(j, KT)
                    qbj, hpj = divmod(bj, NHP)
                    if tj == 0:
                        ctx_cur = (
                            cx_ps.tile([D + 1, SEQT], f32, tag="cx", name="c0"),
                            cx_ps.tile([D + 1, SEQT], f32, tag="cx", name="c1"),
                        )
                    ptj = pts.pop(j)
                    for h in range(2):
                        nc.tensor.matmul(
                            ctx_cur[h][:],
                            lhsT=v_sb[:, tj, 2 * hpj + h, :],
                            rhs=ptj[:, bass.ts(h, SEQT)],
                            start=(tj == 0),
                            stop=(tj == KT - 1),
                        )
                    if tj == KT - 1:
                        def make_part1(bb, cc):
                            def run():
                                qbb, hpb = divmod(bb, NHP)
                                norm_info[bb] = (qbb, hpb) + tuple(
                                    norm_part1(hpb, cc)
                                )
                            return run
                        add(i + 1, make_part1(bj, ctx_cur))
                        def make_part2(bb):
                            def run():
                                norm_part2(*norm_info.pop(bb))
                            return run
                        add(16 * bj + 30, make_part2(bj))
            # tail: block 7's norm + the last q-block's out-proj
            for i in sorted(slots):
                for fn in slots.pop(i, ()):
                    fn()
            for m in range(4):
                outproj_m(QB - 1, m)

    return nc


def _get_program():
    global _PROGRAM
    if _PROGRAM is None:
        _PROGRAM = _build_program()
    return _PROGRAM


def kernel(query, key, value, Wq, bq, Wk, bk, Wv, bv, Wo, bo):
    from concourse.bass_utils import run_bass_kernel_spmd

    nc = _get_program()
    if not getattr(nc, "_waits_split", False):
        _split_excess_waits(nc)
        nc._waits_split = True

    bf = ml_dtypes.bfloat16
    query = np.asarray(query, np.float32)
    key = np.asarray(key, np.float32)
    value = np.asarray(value, np.float32)
    Wq = np.asarray(Wq, np.float32)
    Wk = np.asarray(Wk, np.float32)
    Wv = np.asarray(Wv, np.float32)
    Wo = np.asarray(Wo, np.float32)
    bq = np.asarray(bq, np.float32)
    bk = np.asarray(bk, np.float32)
    bv = np.asarray(bv, np.float32)
    bo = np.asarray(bo, np.float32)

    # Per-batch x^T [E, S] -> [KE, 128, S] bf16
    xT = {}
    for b in range(B):
        xT[("q", b)] = np.ascontiguousarray(query[b].T).astype(bf).reshape(KE, 128, S)
        xT[("k", b)] = np.ascontiguousarray(key[b].T).astype(bf).reshape(KE, 128, S)
        xT[("v", b)] = np.ascontiguousarray(value[b].T).astype(bf).reshape(KE, 128, S)

    in_maps = []
    for c in range(NCORES):
        b = c // HSPLIT
        g = c % HSPLIT
        rsl = slice(DOUT * g, DOUT * (g + 1))
        in_maps.append(
            {
                "xq": xT[("q", b)], "xk": xT[("k", b)], "xv": xT[("v", b)],
                # lhsT for q/k (and rhs for v): (W_g)^T [E, DOUT]
                "wq": np.ascontiguousarray(Wq[rsl, :].T).astype(bf).reshape(KE, 128, DOUT),
                "wk": np.ascontiguousarray(Wk[rsl, :].T).astype(bf).reshape(KE, 128, DOUT),
                "wv": np.ascontiguousarray(Wv[rsl, :].T).astype(bf).reshape(KE, 128, DOUT),
                # rhs for the out-proj: rows g-range of Wo^T as [NHP, 128, E]
                "wo": np.ascontiguousarray(Wo[:, rsl].T).astype(bf).reshape(NHP, 128, E),
                "bq": np.ascontiguousarray(bq[rsl].reshape(NHP, 128).T),
                "bk": np.ascontiguousarray(bk[rsl].reshape(NHP, 128).T),
                "bv": np.ascontiguousarray(bv[rsl].reshape(1, DOUT)).astype(bf),
            }
        )

    res = run_bass_kernel_spmd(nc, in_maps, list(range(NCORES)), trace=False)
    full = np.empty((B, S, E), np.float32)
    for b in range(B):
        acc = np.zeros((S, E), np.float32)
        for g in range(HSPLIT):
            acc += np.asarray(res.results[b * HSPLIT + g]["out"], np.float32)
        full[b] = acc + bo[None, :]
    return full
